# revision 24
# baseline (speedup 1.0000x reference)
"""HGNN+LSTM Bass/Tile kernel for trn2, 8 NeuronCores.

Pipeline per core:
  Phase A (batch-parallel, 4 batches/core): GraphConv aggregations as dense
  adjacency matmuls producing a 32-feature tensor F in node-major layout,
  written to DRAM in bf16.
  AllToAll: reshard F from batch-sharded to node-sharded (13 nodes/core).
  Phase B (node-parallel): fused dense+leakyReLU -> per-node LSTM over T=336
  steps (node-pair-packed matmuls, gates batched across all pairs per step)
  -> linear head + leakyReLU.

Self-contained: hardcodes shapes; includes the TileContext tail-drain patch
needed by this walrus build (one sync-wait per TPB_CTRL instruction).
"""
import sys
sys.path.insert(0, '/opt/trn_rl_repo')

import numpy as np
import ml_dtypes

import concourse.bass as bass
import concourse.tile as tile
from concourse import mybir
from concourse.bass_utils import run_bass_kernel_spmd
from concourse.vector_clock import ScopedClock

F32 = mybir.dt.float32
BF16 = mybir.dt.bfloat16
AF = mybir.ActivationFunctionType

NDEV = 8
B, T, Nh, Nm = 32, 336, 100, 150
Fh, Fm, Hg, Hl, FUT = 8, 16, 64, 64, 24
BL = B // NDEV          # 4 batches per core
GC = BL * T             # 1344 graphs per core
NSH = 13                # real nodes per core
NP = NDEV * NSH         # 104 padded node count
NPAIR = 7               # node pairs per core (last one half-padded)
TC = 56                 # time chunk for phase B
NTC = T // TC           # 6 chunks
NEG = 0.01


# ---------------------------------------------------------------------------
# TileContext tail-drain patch (walrus only accepts 1 sync wait per TPB_CTRL)
def _patched_drain_and_barrier(self, tick_clock, wait_clock):
    import bass_rust
    nc = self.nc
    drain_inst = nc.sync.drain()
    wait_clock.add_sem_waits(
        drain_inst.ins, ScopedClock({None: tick_clock.global_clock})
    )
    waits = list(drain_inst.ins.sync_info.on_wait)
    if len(waits) > 0:
        drain_inst.ins.sync_info.on_wait = []
        for i in range(len(waits)):
            nop = nc.sync.nop(nofuse=True)
            nop.ins.sync_info = bass_rust.SyncInfo(
                on_wait=waits[i:i + 1], on_update=[]
            )
    nc.all_engine_barrier()
    assert self.sems is not None
    popped = nc._tile_sem_poison_stack.pop()
    assert popped is self._sem_poison
    nc.clear_and_free_semaphores(list(self.sems.allocated().values()))
    nc.all_engine_barrier()


tile.TileContext._drain_and_barrier = _patched_drain_and_barrier


# This walrus build accepts only one sync-wait per instruction. Rewrite the
# serialized BIR right before neuronxcc: any instruction carrying k>1 waits
# keeps one and gets k-1 single-wait NoOps inserted in front of it (same
# engine, so the engine's sequencer performs the waits in order).
from concourse import bass2jax as _b2j

_orig_decompress = _b2j._decompress_ant_bir
_fix_ctr = [0]


def _split_sync_waits(bir_bytes):
    import orjson
    d = orjson.loads(bir_bytes)
    changed = False
    for fn in d.get("functions", []):
        for blk in fn.get("blocks", []):
            insts = blk.get("instructions", [])
            new = []
            for ins in insts:
                si = ins.get("sync_info") or {}
                waits = si.get("on_wait") or []
                keep = 0 if ins.get("opcode") == "Drain" else 1
                if len(waits) > keep:
                    changed = True
                    head, tail = waits[:keep], waits[keep:]
                    for w in tail:
                        _fix_ctr[0] += 1
                        new.append({
                            "debug": ins.get("debug", 0),
                            "engine": ins["engine"],
                            "ins": [], "outs": [],
                            "is_reset_sema": False,
                            "name": f"I-waitfix-{_fix_ctr[0]}",
                            "opcode": "NoOp",
                            "sync_info": {"on_update": [], "on_wait": [w]},
                        })
                    si = dict(si)
                    si["on_wait"] = head
                    ins = dict(ins)
                    ins["sync_info"] = si
                new.append(ins)
            blk["instructions"] = new
    if changed:
        return orjson.dumps(d)
    return bir_bytes


def _patched_decompress(ant_bir_value):
    return _split_sync_waits(_orig_decompress(ant_bir_value))


_b2j._decompress_ant_bir = _patched_decompress
# ---------------------------------------------------------------------------


_U16_ITEMS = [("AhT", 128 * NP), ("ImT", 128 * NP), ("AmTa", 128 * NP),
              ("AmTb", 32 * NP), ("WcatP", 64 * 128), ("WlinT", 64 * FUT),
              ("WihP", 128 * NPAIR * 256), ("WhhP", 128 * NPAIR * 2 * 128)]
_F32_ITEMS = [("bcat2", 128), ("blin", FUT), ("biasP", 128 * NPAIR * 2 * 2)]
U16_OFFS, F32_OFFS = {}, {}
_o = 0
for _k, _s in _U16_ITEMS:
    U16_OFFS[_k] = (_o, _s)
    _o += _s
U16_TOTAL = _o
_o = 0
for _k, _s in _F32_ITEMS:
    F32_OFFS[_k] = (_o, _s)
    _o += _s
F32_TOTAL = _o


def _build_nc():
    nc = bass.Bass()
    dhT = nc.declare_dram_parameter("dhT", [Nh, GC, Fh], F32, isOutput=False)
    dmT = nc.declare_dram_parameter("dmT", [Nm, GC, Fm], F32, isOutput=False)
    # all bf16 weights packed into one uint16 blob; fp32 biases into another
    wblob = nc.declare_dram_parameter("wblob", [U16_TOTAL], mybir.dt.uint16,
                                      isOutput=False)
    fblob = nc.declare_dram_parameter("fblob", [F32_TOTAL], F32, isOutput=False)

    def bfv(key, shape):
        off, sz = U16_OFFS[key]
        ap = wblob[off:off + sz].bitcast(BF16)
        pat = {2: "(p a) -> p a", 3: "(p a b) -> p a b",
               4: "(p a b c) -> p a b c"}[len(shape)]
        kw = dict(zip("pab", shape[:-1]))
        return ap.rearrange(pat, **kw)

    def f32v(key, shape):
        off, sz = F32_OFFS[key]
        pat = {2: "(p a) -> p a", 3: "(p a b) -> p a b",
               4: "(p a b c) -> p a b c"}[len(shape)]
        kw = dict(zip("pab", shape[:-1]))
        return fblob[off:off + sz].rearrange(pat, **kw)

    AhT = bfv("AhT", (128, NP))
    ImT = bfv("ImT", (128, NP))
    AmTa = bfv("AmTa", (128, NP))
    AmTb = bfv("AmTb", (32, NP))
    WcatP = bfv("WcatP", (64, 128))
    WlinT = bfv("WlinT", (64, FUT))
    WihP = bfv("WihP", (128, NPAIR, 256))
    WhhP = bfv("WhhP", (128, NPAIR, 2, 128))
    bcat2 = f32v("bcat2", (128, 1))
    blin = f32v("blin", (FUT, 1))
    biasP = f32v("biasP", (128, NPAIR, 2, 2))
    out = nc.declare_dram_parameter("out", [FUT, NPAIR * 2 * B], F32, isOutput=True)

    with tile.TileContext(nc) as tc:
        with tc.tile_pool(name="dram", bufs=1, space="DRAM") as dram, \
             tc.tile_pool(name="wpool", bufs=1) as wpool:
            a2a_in = dram.tile([NP, 32, GC], BF16)
            a2a_out = dram.tile([NP, 32, GC], BF16)

            # ---------------- Phase A: GNN aggregations -> F ----------------
            with tc.tile_pool(name="apool", bufs=1) as apool, \
                 tc.tile_pool(name="apsum", bufs=2, space="PSUM") as apsum:
                AhT_s = wpool.tile([128, NP], BF16, tag="AhT")
                nc.sync.dma_start(AhT_s[:], AhT[:])
                ImT_s = wpool.tile([128, NP], BF16, tag="ImT")
                nc.sync.dma_start(ImT_s[:], ImT[:])
                AmTa_s = wpool.tile([128, NP], BF16, tag="AmTa")
                nc.sync.dma_start(AmTa_s[:], AmTa[:])
                AmTb_s = wpool.tile([32, NP], BF16, tag="AmTb")
                nc.sync.dma_start(AmTb_s[:], AmTb[:])

                xhT = apool.tile([128, GC, Fh], BF16)
                nc.gpsimd.memset(xhT[:], 0.0)
                nc.gpsimd.dma_start(xhT[0:100], dhT[:])
                xmTa = apool.tile([128, GC, Fm], BF16)
                nc.gpsimd.dma_start(xmTa[:], dmT[0:128])
                xmTb = apool.tile([32, GC, Fm], BF16)
                nc.gpsimd.memset(xmTb[:], 0.0)
                nc.gpsimd.dma_start(xmTb[0:22], dmT[128:150])

                Fb = apool.tile([NP, 32, GC], BF16)
                GCH = 32
                for gi in range(GC // GCH):
                    g0 = gi * GCH
                    ph = apsum.tile([NP, GCH, Fh], F32, tag="ph")
                    nc.tensor.matmul(ph[:], AhT_s[:], xhT[:, g0:g0 + GCH, :],
                                     start=True, stop=True)
                    pr = apsum.tile([NP, GCH, Fh], F32, tag="pr")
                    nc.tensor.matmul(pr[:], ImT_s[:], xhT[:, g0:g0 + GCH, :],
                                     start=True, stop=True)
                    pm = apsum.tile([NP, GCH, Fm], F32, tag="pm")
                    nc.tensor.matmul(pm[:], AmTa_s[:], xmTa[:, g0:g0 + GCH, :],
                                     start=True, stop=False)
                    nc.tensor.matmul(pm[:], AmTb_s[:], xmTb[:, g0:g0 + GCH, :],
                                     start=False, stop=True)
                    # copy (g,f) psum -> (f,g) F slices, cast to bf16
                    nc.vector.tensor_copy(Fb[:, 0:Fh, g0:g0 + GCH],
                                          ph.rearrange("p g f -> p f g"))
                    nc.scalar.copy(Fb[:, Fh:2 * Fh, g0:g0 + GCH],
                                   pr.rearrange("p g f -> p f g"))
                    nc.vector.tensor_copy(Fb[:, 2 * Fh:32, g0:g0 + GCH],
                                          pm.rearrange("p g f -> p f g"))
                nc.sync.dma_start(a2a_in[:], Fb[:])

            nc.gpsimd.collective_compute(
                "AllToAll", mybir.AluOpType.bypass,
                replica_groups=[list(range(NDEV))],
                ins=[a2a_in[:]], outs=[a2a_out[:]],
            )

            # ---------------- Phase B: dense + LSTM + head ----------------
            with tc.tile_pool(name="bpool", bufs=1) as bpool, \
                 tc.tile_pool(name="bwork", bufs=2) as bwork, \
                 tc.tile_pool(name="bpsum", bufs=2, space="PSUM") as bpsum, \
                 tc.tile_pool(name="gpsum", bufs=1, space="PSUM") as gpsum:
                WihP_s = wpool.tile([128, NPAIR, 256], BF16, tag="WihP")
                nc.sync.dma_start(WihP_s[:], WihP[:])
                WhhP_s = wpool.tile([128, NPAIR, 2, 128], BF16, tag="WhhP")
                nc.sync.dma_start(WhhP_s[:], WhhP[:])
                WcatP_s = wpool.tile([64, 128], BF16, tag="WcatP")
                nc.sync.dma_start(WcatP_s[:], WcatP[:])
                bcat2_s = wpool.tile([128, 1], F32, tag="bcat2")
                nc.sync.dma_start(bcat2_s[:], bcat2[:])
                biasP_s = wpool.tile([128, NPAIR, 2, 2], F32, tag="biasP")
                nc.sync.dma_start(biasP_s[:], biasP[:])
                WlinT_s = wpool.tile([64, FUT], BF16, tag="WlinT")
                nc.sync.dma_start(WlinT_s[:], WlinT[:])
                blin_s = wpool.tile([FUT, 1], F32, tag="blin")
                nc.sync.dma_start(blin_s[:], blin[:])

                gx = bpool.tile([128, 2, TC, 448], BF16)
                h_bd = bpool.tile([128, NPAIR, 64], BF16)
                nc.vector.memset(h_bd[:], 0.0)
                cst = bpool.tile([128, 448], F32)   # c state in rows 64:128
                nc.vector.memset(cst[:], 0.0)
                h_flat = bpool.tile([64, 448], BF16)

                # group split: A = pairs 0..3 (cols 0:256), B = pairs 4..6
                groups = [(0, 4, 0), (4, 3, 256)]
                psGA = gpsum.tile([128, 2, 256], F32)
                psGB = gpsum.tile([128, 2, 192], F32)
                psG = {0: psGA, 4: psGB}

                a2a_view = a2a_out.rearrange("(j l) f (b t) -> l f j b t",
                                             l=NSH, b=BL)

                for tci in range(NTC):
                    t0 = tci * TC
                    # ---- gx production for this time chunk ----
                    for p in range(NPAIR):
                        Ftc = bwork.tile([64, NDEV, BL, TC], BF16, tag="Ftc")
                        if 2 * p + 1 >= NSH:
                            nc.vector.memset(Ftc[:], 0.0)
                        for j in range(NDEV):
                            nc.sync.dma_start(
                                Ftc[0:32, j],
                                a2a_view[2 * p, :, j, :, t0:t0 + TC])
                            if 2 * p + 1 < NSH:
                                nc.sync.dma_start(
                                    Ftc[32:64, j],
                                    a2a_view[2 * p + 1, :, j, :, t0:t0 + TC])
                        Ff = Ftc.rearrange("p j b t -> p (j b t)")
                        xtc = bwork.tile([128, B * TC], BF16, tag="xtc")
                        for ci in range(4):
                            c0 = ci * 448
                            psx = bpsum.tile([128, 448], F32, tag="psx")
                            nc.tensor.matmul(psx[:], WcatP_s[:], Ff[:, c0:c0 + 448],
                                             start=True, stop=True)
                            nc.scalar.activation(xtc[:, c0:c0 + 448], psx[:],
                                                 AF.Lrelu, bias=bcat2_s[:],
                                                 alpha=NEG)
                        for h in range(2):
                            for ni in range(2):
                                lhsT = WihP_s[64 * ni:64 * (ni + 1), p,
                                              128 * h:128 * (h + 1)]
                                bap = biasP_s[:, p, h, ni:ni + 1]
                                for bi in range(8):
                                    psg = bpsum.tile([128, 224], F32, tag="psg")
                                    nc.tensor.matmul(
                                        psg[:], lhsT,
                                        xtc[64 * ni:64 * (ni + 1),
                                            bi * 224:(bi + 1) * 224],
                                        start=True, stop=True)
                                    dst = gx[:, h, :,
                                             p * 64 + ni * 32 + bi * 4:
                                             p * 64 + ni * 32 + bi * 4 + 4]
                                    dst = dst.rearrange("p t b -> p b t")
                                    src = psg.rearrange("p (b t) -> p b t", b=4)
                                    if bi % 2 == 0:
                                        nc.scalar.activation(dst, src,
                                                             AF.Identity,
                                                             bias=bap)
                                    else:
                                        nc.vector.tensor_scalar_add(dst, src, bap)

                    # ---- recurrence over this chunk ----
                    for tl in range(TC):
                        for (p0, npair, col0) in groups:
                            w = npair * 64
                            ps = psG[p0]
                            for pi in range(npair):
                                p = p0 + pi
                                nc.tensor.matmul(ps[:, 0, pi * 64:(pi + 1) * 64],
                                                 WhhP_s[:, p, 0, :],
                                                 h_bd[:, p, :],
                                                 start=True, stop=True)
                                nc.tensor.matmul(ps[:, 1, pi * 64:(pi + 1) * 64],
                                                 WhhP_s[:, p, 1, :],
                                                 h_bd[:, p, :],
                                                 start=True, stop=True)
                            tg = bwork.tile([128, 2, 256], BF16, tag=f"tg_{p0}")
                            nc.vector.tensor_add(tg[:, :, 0:w], ps[:, :, :],
                                                 gx[:, :, tl, col0:col0 + w])
                            tg0 = tg[:, 0]
                            tg1 = tg[:, 1]
                            s_if = bwork.tile([128, 448], BF16, tag=f"sif_{p0}")
                            nc.scalar.activation(s_if[:, 0:w], tg0[:, 0:w],
                                                 AF.Sigmoid)
                            t_g = bwork.tile([64, 448], BF16, tag=f"tg2_{p0}")
                            nc.scalar.activation(t_g[:, 0:w], tg1[0:64, 0:w],
                                                 AF.Tanh)
                            s_o = bwork.tile([64, 448], BF16, tag=f"so_{p0}")
                            nc.scalar.activation(s_o[:, 0:w], tg1[64:128, 0:w],
                                                 AF.Sigmoid)
                            c = cst[64:128, col0:col0 + w]
                            nc.vector.tensor_mul(c, c, s_if[64:128, 0:w])
                            tmp = bwork.tile([128, 448], F32, tag=f"tmp_{p0}")
                            nc.vector.tensor_mul(tmp[64:128, 0:w],
                                                 s_if[0:64, 0:w], t_g[:, 0:w])
                            nc.vector.tensor_add(c, c, tmp[64:128, 0:w])
                            th_c = bwork.tile([64, 448], BF16, tag=f"thc_{p0}")
                            nc.scalar.activation(th_c[:, 0:w], c, AF.Tanh)
                            so_v = s_o.rearrange("p (q n b) -> p q n b", n=2, b=32)
                            th_v = th_c.rearrange("p (q n b) -> p q n b", n=2, b=32)
                            last = (tci == NTC - 1 and tl == TC - 1)
                            hdst_e = h_bd[0:64, p0:p0 + npair, 0:32]
                            hdst_o = h_bd[64:128, p0:p0 + npair, 32:64]
                            if last:
                                hfv = h_flat.rearrange("p (q n b) -> p q n b",
                                                       n=2, b=32)
                                nc.vector.tensor_mul(
                                    hfv[:, p0:p0 + npair, 0, :],
                                    so_v[:, 0:npair, 0, :], th_v[:, 0:npair, 0, :])
                                nc.vector.tensor_mul(
                                    hfv[:, p0:p0 + npair, 1, :],
                                    so_v[:, 0:npair, 1, :], th_v[:, 0:npair, 1, :])
                            else:
                                nc.vector.tensor_mul(hdst_e, so_v[:, 0:npair, 0, :],
                                                     th_v[:, 0:npair, 0, :])
                                nc.vector.tensor_mul(hdst_o, so_v[:, 0:npair, 1, :],
                                                     th_v[:, 0:npair, 1, :])

                # head: pred = lrelu(W_lin @ h + b_lin)
                psH = bpsum.tile([FUT, 448], F32, tag="psx")
                nc.tensor.matmul(psH[:], WlinT_s[:], h_flat[:],
                                 start=True, stop=True)
                out_s = bpool.tile([FUT, 448], F32)
                nc.scalar.activation(out_s[:], psH[:], AF.Lrelu,
                                     bias=blin_s[:], alpha=NEG)
                nc.sync.dma_start(out[:], out_s[:])
    return nc


_NC_CACHE = None


def _get_nc():
    global _NC_CACHE
    if _NC_CACHE is None:
        _NC_CACHE = _build_nc()
    return _NC_CACHE


def _host_prep(inputs):
    bf = ml_dtypes.bfloat16
    ei_h = np.asarray(inputs['hydro_edge_index'])
    ei_m = np.asarray(inputs['meteo_edge_index'])
    A_h = np.zeros((Nh, Nh), np.float32)
    np.add.at(A_h, (ei_h[1], ei_h[0]), 1.0)
    A_m = np.zeros((Nh, Nm), np.float32)
    np.add.at(A_m, (ei_m[1], ei_m[0]), 1.0)

    AhT = np.zeros((128, NP), np.float32)
    AhT[0:Nh, 0:Nh] = A_h.T
    ImT = np.zeros((128, NP), np.float32)
    ImT[0:Nh, 0:Nh] = np.eye(Nh)
    AmT = A_m.T  # [150, 100]
    AmTa = np.zeros((128, NP), np.float32)
    AmTa[:, 0:Nh] = AmT[0:128]
    AmTb = np.zeros((32, NP), np.float32)
    AmTb[0:22, 0:Nh] = AmT[128:150]

    W_rel_h = np.asarray(inputs['W_rel_h'])
    W_rel_m = np.asarray(inputs['W_rel_m'])
    W_root = np.asarray(inputs['W_root_h']) + np.asarray(inputs['W_root_m'])
    Wcat = 0.5 * np.concatenate([W_rel_h.T, W_root.T, W_rel_m.T], axis=0)  # [32, 64]
    WcatP = np.zeros((64, 128), np.float32)
    WcatP[0:32, 0:64] = Wcat
    WcatP[32:64, 64:128] = Wcat
    bcat = 0.5 * (np.asarray(inputs['b_rel_h']) + np.asarray(inputs['b_rel_m']))
    bcat2 = np.tile(bcat, 2).reshape(128, 1).astype(np.float32)

    W_ih = np.asarray(inputs['W_ih'])   # [100, 256, 64]
    W_hh = np.asarray(inputs['W_hh'])   # [100, 256, 64]
    bias = (np.asarray(inputs['b_ih']) + np.asarray(inputs['b_hh']))  # [100, 256]
    WlinT = np.asarray(inputs['W_lin']).T  # [64, 24]
    blin = np.asarray(inputs['b_lin']).reshape(FUT, 1).astype(np.float32)

    dh_bf = np.asarray(inputs['data_hydro'], np.float32)  # [B, T, Nh, Fh]
    dm_bf = np.asarray(inputs['data_meteo'], np.float32)  # [B, T, Nm, Fm]

    def u16(a):
        return np.ascontiguousarray(a).astype(bf).view(np.uint16).ravel()

    common_u16 = np.concatenate([
        u16(AhT), u16(ImT), u16(AmTa), u16(AmTb), u16(WcatP), u16(WlinT)])

    in_maps = []
    for c in range(NDEV):
        nodes = np.clip(np.arange(NSH * c, NSH * c + NPAIR * 2), 0, Nh - 1)
        WihP = np.zeros((128, NPAIR, 256), np.float32)
        WhhP = np.zeros((128, NPAIR, 2, 128), np.float32)
        biasP = np.zeros((128, NPAIR, 2, 2), np.float32)
        for p in range(NPAIR):
            for ni in range(2):
                n = nodes[2 * p + ni]
                WihP[64 * ni:64 * (ni + 1), p, :] = W_ih[n].T  # [64, 256]
                for h in range(2):
                    WhhP[64 * ni:64 * (ni + 1), p, h, :] = \
                        W_hh[n][128 * h:128 * (h + 1), :].T
                    biasP[:, p, h, ni] = bias[n][128 * h:128 * (h + 1)]
        m = {}
        m['dhT'] = np.ascontiguousarray(
            dh_bf[BL * c:BL * (c + 1)].reshape(GC, Nh, Fh).transpose(1, 0, 2))
        m['dmT'] = np.ascontiguousarray(
            dm_bf[BL * c:BL * (c + 1)].reshape(GC, Nm, Fm).transpose(1, 0, 2))
        m['wblob'] = np.concatenate([common_u16, u16(WihP), u16(WhhP)])
        m['fblob'] = np.concatenate([
            bcat2.ravel(), blin.ravel(), biasP.ravel()]).astype(np.float32)
        assert m['wblob'].size == U16_TOTAL and m['fblob'].size == F32_TOTAL
        in_maps.append(m)
    return in_maps


def kernel(**inputs):
    nc = _get_nc()
    in_maps = _host_prep(inputs)
    res = run_bass_kernel_spmd(nc, in_maps, list(range(NDEV)))
    pred = np.zeros((B, Nh, FUT), np.float32)
    for c in range(NDEV):
        o = res.results[c]['out'].reshape(FUT, NPAIR, 2, B)
        for l in range(NSH):
            n = NSH * c + l
            if n < Nh:
                pred[:, n, :] = o[:, l // 2, l % 2, :].T
    return pred
